# revision 1
# baseline (speedup 1.0000x reference)
"""Causal self-attention (B=8, T=2048, C=256, H=8, D=32) on 8 trn2 NeuronCores.

Sharding: pure data-parallel over batch — core b computes batch element b
end-to-end (no collectives).

Per-core kernel design (all matmul inputs bf16, fp32 PSUM accumulation,
softmax math in fp32):
  1. x [T,256] fp32 -> cast bf16 -> DMA-transpose -> xT [256,T] bf16 (2 tiles)
  2. qT,kT [256,T]: PE matmuls lhsT=w_attn slice, rhs=xT  (transposed layout)
     v   [T,256] natural: lhsT=xT slice, rhs=w_attn v-cols; stored as
     v_aug [128,8*33] per t-tile: per-head 32 v-cols + a ones column (the
     ones column makes the PV matmul accumulate the softmax denominator).
  3. Attention in S^T layout: for head-group g (4 heads), m-chunk (512 wide),
     n-tile (128 keys): S^T[n, 4*512 m] = 4 row-tiled concurrent K=32 matmuls
     (tile_position=(32h,0)) into one 4-bank PSUM tile; ONE big ACT exp
     (scale=1/sqrt(D) fused, fp32, no max-subtraction needed since |s|<~10)
     PSUM->SBUF bf16; triangular bf16 mask multiply on diagonal blocks;
     PV: per head pair col-tiled M=33 matmuls (32 y rows + denom row)
     accumulating y_aug^T [33,512] over n-tiles.  Causality: above-diagonal
     n-tiles are skipped entirely; diagonal tiles slice columns >= 128j.
  4. Normalize: reciprocal of denom row, DMA partition-broadcast, DVE mult
     -> yT [256,T] bf16.  5. proj: out = y @ w_proj -> PSUM -> DMA to DRAM.
"""

import numpy as np
from contextlib import ExitStack

import concourse.bass as bass
import concourse.bacc as bacc
import concourse.mybir as mybir
import concourse.tile as tile
from concourse.bass import ds
from concourse.bass_utils import run_bass_kernel_spmd
from concourse.masks import make_upper_triangular, make_identity
from concourse import library_config

FP32 = mybir.dt.float32
BF16 = mybir.dt.bfloat16

C = 256
H = 8
D = 32
N_CORES = 8
SCALE = 1.0 / float(np.sqrt(np.float32(D)))


def build_body(ctx: ExitStack, tc: tile.TileContext, x, wa, wp, out, T: int):
    nc = tc.nc
    TT = T // 128              # number of 128-row t-tiles
    W = min(512, T)            # m-chunk width
    MCN = T // W               # number of m-chunks
    WT = W // 128              # n-tiles per m-chunk width

    # NOTE: each distinct tag in a pool gets `bufs` slots; persistent tiles
    # use distinct tags with bufs=1, cycling tiles share one tag.
    const = ctx.enter_context(tc.tile_pool(name="const", bufs=1))
    wpool = ctx.enter_context(tc.tile_pool(name="wpool", bufs=1))
    wstage = ctx.enter_context(tc.tile_pool(name="wstage", bufs=4))
    # fresh slot per x t-tile: keeps every HWDGE input DMA at zero waits
    # (walrus rejects HWDGE DMAs with >1 sem wait)
    xload = ctx.enter_context(tc.tile_pool(name="xload", bufs=TT))
    xbfp = ctx.enter_context(tc.tile_pool(name="xbfp", bufs=3))
    xTp = ctx.enter_context(tc.tile_pool(name="xTp", bufs=1))
    qkTp = ctx.enter_context(tc.tile_pool(name="qkTp", bufs=1))
    vaugp = ctx.enter_context(tc.tile_pool(name="vaugp", bufs=TT))
    ptp = ctx.enter_context(tc.tile_pool(name="ptp", bufs=2))
    ytp = ctx.enter_context(tc.tile_pool(name="ytp", bufs=1))
    nrmp = ctx.enter_context(tc.tile_pool(name="nrmp", bufs=4))

    # PSUM budget (8 banks of [128, 2KB]):
    #   ps_s: 1 tag x 1 buf x [128, 4W] fp32 = 4 banks
    #   ps_y: 2 tags x 1 buf x [128, W] fp32 = 2 banks
    #   ps_sm: 1 shared tag x 2 bufs x <=1 bank = 2 banks
    ps_s = ctx.enter_context(tc.tile_pool(name="ps_s", bufs=1, space="PSUM"))
    ps_y = ctx.enter_context(tc.tile_pool(name="ps_y", bufs=1, space="PSUM"))
    ps_sm = ctx.enter_context(tc.tile_pool(name="ps_sm", bufs=2, space="PSUM"))

    # --- constants: triangular keep-mask (n' <= m'), replicated 4x ---
    tri = const.tile([128, 128], BF16)
    make_upper_triangular(nc, tri[:], val=1.0, diag=True)
    tri4 = const.tile([128, 512], BF16)
    for i in range(4):
        nc.vector.tensor_copy(tri4[:, 128 * i:128 * (i + 1)], tri[:])
    tri4v = tri4[:].rearrange("p (h m) -> p h m", h=4)
    # ones row [1, 32] for the K=1 broadcast matmuls
    ones1 = const.tile([1, D], BF16)
    nc.gpsimd.memset(ones1[:], 1.0)
    # identity for PE transposes
    ident = const.tile([128, 128], BF16)
    make_identity(nc, ident[:])

    # --- weights: load fp32, cast to bf16 ---
    wa_bf = []
    wp_bf = []
    for k in range(2):
        wa_f = wstage.tile([128, 3 * C], FP32, name=f"wa_f{k}", tag="wstage")
        nc.sync.dma_start(wa_f[:], wa[128 * k:128 * (k + 1), :])
        wab = wpool.tile([128, 3 * C], BF16, name=f"wa_bf{k}")
        nc.vector.tensor_copy(wab[:], wa_f[:])
        wa_bf.append(wab)
        wp_f = wstage.tile([128, C], FP32, name=f"wp_f{k}", tag="wstage")
        nc.sync.dma_start(wp_f[:], wp[128 * k:128 * (k + 1), :])
        wpb = wpool.tile([128, C], BF16, name=f"wp_bf{k}")
        nc.vector.tensor_copy(wpb[:], wp_f[:])
        wp_bf.append(wpb)

    # --- x: load, cast bf16, DMA-transpose into xT [256, T] (2 tiles) ---
    xT = [xTp.tile([128, T], BF16, name=f"xT{k}") for k in range(2)]
    x_bfs = []
    for tt in range(TT):
        x_f = xload.tile([128, C], FP32, name="x_f")
        nc.sync.dma_start(x_f[:], x[128 * tt:128 * (tt + 1), :])
        x_bf = xbfp.tile([128, C], BF16, name="x_bf")
        nc.vector.tensor_copy(x_bf[:], x_f[:])
        x_bfs.append(x_bf)
        for k in range(2):
            tp_ps = ps_sm.tile([128, 128], BF16, name="tp_ps", tag="sm")
            nc.tensor.transpose(tp_ps[:], x_bf[:, 128 * k:128 * (k + 1)],
                                ident[:])
            nc.vector.tensor_copy(xT[k][:, 128 * tt:128 * (tt + 1)], tp_ps[:])

    # --- qT, kT: [256, T] each as 2 f-tiles of [128, T] ---
    # global f-tile f in 0..3: q rows 0:256 (f=0,1), k rows 256:512 (f=2,3)
    qkT = [qkTp.tile([128, T], BF16, name=f"qkT{f}") for f in range(4)]
    for f in range(4):
        for tck in range(MCN):
            ps = ps_sm.tile([128, W], FP32, name="qk_ps", tag="sm")
            for k in range(2):
                nc.tensor.matmul(
                    ps[:],
                    wa_bf[k][:, 128 * f:128 * (f + 1)],
                    xT[k][:, W * tck:W * (tck + 1)],
                    start=(k == 0),
                    stop=(k == 1),
                )
            nc.vector.tensor_copy(qkT[f][:, W * tck:W * (tck + 1)], ps[:])

    # --- v natural + ones column -> v_aug [128, 8*33] per t-tile ---
    vaug = []
    for tt in range(TT):
        ps = ps_sm.tile([128, C], FP32, name="v_ps", tag="sm")
        for k in range(2):
            nc.tensor.matmul(
                ps[:],
                xT[k][:, 128 * tt:128 * (tt + 1)],
                wa_bf[k][:, 2 * C:3 * C],
                start=(k == 0),
                stop=(k == 1),
            )
        va = vaugp.tile([128, H * (D + 1)], BF16, name="va")
        nc.gpsimd.memset(va[:], 1.0)
        nc.vector.tensor_copy(
            va[:].rearrange("p (h d) -> p h d", h=H)[:, :, 0:D],
            ps[:].rearrange("p (h d) -> p h d", h=H),
        )
        vaug.append(va)

    # --- attention + projection, per m-chunk ---
    yT = [ytp.tile([128, T], BF16, name=f"yT{g}") for g in range(2)]
    for mc in range(MCN):
        for g in range(2):
            qt = qkT[g]       # q features for heads 4g..4g+3
            kt = qkT[2 + g]   # k features
            nn_count = WT * (mc + 1)
            nn_last = nn_count - 1
            y_ps = [ps_y.tile([128, W], FP32, name=f"y_ps{b}") for b in range(2)]
            for nn in range(nn_count):
                j = nn - WT * mc  # >= 0 on diagonal-crossing tiles
                s_ps = ps_s.tile([128, 4 * W], FP32, name="s_ps")
                for hh in range(4):
                    nc.tensor.matmul(
                        s_ps[:, W * hh:W * (hh + 1)],
                        kt[32 * hh:32 * (hh + 1), 128 * nn:128 * (nn + 1)],
                        qt[32 * hh:32 * (hh + 1), W * mc:W * (mc + 1)],
                        start=True,
                        stop=True,
                        tile_position=(32 * hh, 0),
                        skip_group_check=True,
                    )
                pt = ptp.tile([128, 4 * W], BF16, name="pt")
                off = max(0, 128 * j)
                sv = s_ps[:].rearrange("p (h m) -> p h m", h=4)
                pv = pt[:].rearrange("p (h m) -> p h m", h=4)
                nc.scalar.activation(
                    pv[:, :, ds(off, W - off)],
                    sv[:, :, ds(off, W - off)],
                    mybir.ActivationFunctionType.Exp,
                    scale=SCALE,
                )
                if j >= 0:
                    # triangular mask on the [128,128] diagonal block of each head
                    blk = pv[:, :, ds(off, 128)]
                    nc.vector.tensor_mul(blk, blk, tri4v[:, :, 0:128])
                for b in range(2):
                    for i in range(2):
                        hloc = 2 * b + i
                        hg = 4 * g + hloc
                        nc.tensor.matmul(
                            y_ps[b][ds(64 * i, D + 1), ds(off, W - off)],
                            vaug[nn][:, (D + 1) * hg:(D + 1) * hg + (D + 1)],
                            pt[:, W * hloc + off:W * (hloc + 1)],
                            start=(nn == 0),
                            stop=(nn == nn_last),
                            tile_position=(0, 64 * i),
                            skip_group_check=True,
                        )
            # normalize: yT[g][64b+32i : +32, mc cols] = y / denom.
            # recip broadcast over 32 partitions via K=2 bf16 indicator
            # matmuls; full fp32 precision kept with r = hi + lo split.
            for b in range(2):
                for i in range(2):
                    rcp = nrmp.tile([1, W], FP32, name="rcp")
                    nc.vector.reciprocal(
                        rcp[:], y_ps[b][64 * i + D:64 * i + D + 1, :])
                    r_hi = nrmp.tile([1, W], BF16, name="r_hi")
                    nc.vector.tensor_copy(r_hi[:], rcp[:])
                    r_lo = nrmp.tile([1, W], BF16, name="r_lo")
                    nc.vector.tensor_sub(r_lo[:], rcp[:], r_hi[:])
                    bc_ps = ps_sm.tile([D, W], FP32, name="bc_ps", tag="sm")
                    nc.tensor.matmul(bc_ps[:], ones1[:], r_hi[:],
                                     start=True, stop=False)
                    nc.tensor.matmul(bc_ps[:], ones1[:], r_lo[:],
                                     start=False, stop=True)
                    bcast = nrmp.tile([D, W], FP32, name="bcast")
                    nc.vector.tensor_copy(bcast[:], bc_ps[:])
                    nc.vector.tensor_mul(
                        yT[g][64 * b + 32 * i:64 * b + 32 * i + 32, W * mc:W * (mc + 1)],
                        y_ps[b][64 * i:64 * i + D, :],
                        bcast[:],
                    )
        # projection for this m-chunk's t-tiles
        for tt in range(WT * mc, WT * (mc + 1)):
            ps = ps_sm.tile([128, C], FP32, name="pj_ps", tag="sm")
            for g in range(2):
                nc.tensor.matmul(
                    ps[:],
                    yT[g][:, 128 * tt:128 * (tt + 1)],
                    wp_bf[g][:],
                    start=(g == 0),
                    stop=(g == 1),
                )
            ost = xbfp.tile([128, C], FP32, name="ost", tag="ost")
            nc.vector.tensor_copy(ost[:], ps[:])
            # SWDGE: out-store waits are executed by Q7 ucode (no 1-wait cap)
            nc.gpsimd.dma_start(out[128 * tt:128 * (tt + 1), :], ost[:])


def build_nc(T: int = 2048) -> bass.Bass:
    # Bacc (not raw Bass): its compile() pass legalizes multi-sem waits via
    # event semaphores — walrus only accepts one sem wait per instruction.
    nc = bacc.Bacc("TRN2", target_bir_lowering=False, debug=False,
                   num_devices=N_CORES)
    x_d = nc.dram_tensor("x", [T, C], FP32, kind="ExternalInput")
    wa_d = nc.dram_tensor("w_attn", [C, 3 * C], FP32, kind="ExternalInput")
    wp_d = nc.dram_tensor("w_proj", [C, C], FP32, kind="ExternalInput")
    out_d = nc.dram_tensor("out", [T, C], FP32, kind="ExternalOutput")
    with tile.TileContext(nc) as tc:
        with ExitStack() as ctx:
            build_body(ctx, tc, x_d.ap(), wa_d.ap(), wp_d.ap(), out_d.ap(), T)
    nc.compile()
    return nc


_NC_CACHE: dict[int, bass.Bass] = {}


def _get_nc(T: int) -> bass.Bass:
    if T not in _NC_CACHE:
        _NC_CACHE[T] = build_nc(T)
    return _NC_CACHE[T]


def kernel(x: np.ndarray, w_attn: np.ndarray, w_proj: np.ndarray,
           **run_kwargs) -> np.ndarray:
    B, T, C_ = x.shape
    assert B == N_CORES and C_ == C
    nc = _get_nc(T)
    wa = np.ascontiguousarray(w_attn, dtype=np.float32)
    wp = np.ascontiguousarray(w_proj, dtype=np.float32)
    in_maps = [
        {"x": np.ascontiguousarray(x[b], dtype=np.float32), "w_attn": wa,
         "w_proj": wp}
        for b in range(B)
    ]
    res = run_bass_kernel_spmd(nc, in_maps, list(range(N_CORES)), **run_kwargs)
    out = np.stack([res.results[b]["out"] for b in range(B)])
    return out.astype(np.float32)



# revision 6
# speedup vs baseline: 1.6393x; 1.6393x over previous
"""Causal self-attention (B=8, T=2048, C=256, H=8, D=32) on 8 trn2 NeuronCores.

Sharding: pure data-parallel over batch - core b computes batch element b
end-to-end (no collectives).

Per-core kernel v2 - restructured so the scalar (ACT) engine, which owns the
irreducible exp work (~116us of element time), runs at ~100% duty:

  1. Pipelined prologue: x is processed in 512-column chunks (load, cast,
     PE-transpose, qkT chunk, vaug tiles), emitted so chunk c+1's prologue
     overlaps chunk c's attention.
  2. Attention in S^T layout per (head-group g of 4, m-chunk of 512):
     the score PSUM is split into TWO 2-bank tiles (heads 0-1 / heads 2-3)
     and the softmax exp ACT is split into matching halves, so the next
     n-tile's S matmuls (WAR on the score PSUM) can start as soon as the
     first half's ACT is done, hiding the S matmuls + semaphore handoff
     under the second ACT half. PV matmuls for step nn are emitted AFTER
     step nn+1's S matmuls so the PE queue never blocks the S->ACT chain.
  3. Causal masking: above-diagonal n-tiles skipped; diagonal 128-blocks
     get a triangular bf16 mask-multiply on the exp output (DVE, off the
     critical chain).
  4. PV via the 33-column v_aug trick (32 v-cols + ones column accumulates
     the softmax denominator) into 2 PSUM banks; after the last n-tile the
     y_aug PSUM is copied to SBUF immediately (frees banks for the next
     group) and normalization runs there: per-group batched reciprocal
     [4,512] (one instr instead of 4x [1,512] - 8x less DVE time),
     hi/lo-split K=1 broadcast matmuls, DVE multiply into yT bf16.
  5. proj per t-tile -> PSUM -> SWDGE DMA to DRAM, emitted per chunk.
"""

import numpy as np
from contextlib import ExitStack

import concourse.bass as bass
import concourse.bacc as bacc
import concourse.mybir as mybir
import concourse.tile as tile
from concourse.bass import ds
from concourse.bass_utils import run_bass_kernel_spmd
from concourse.masks import make_upper_triangular, make_identity
from concourse import library_config

FP32 = mybir.dt.float32
BF16 = mybir.dt.bfloat16

C = 256
H = 8
D = 32
N_CORES = 8
SCALE = 1.0 / float(np.sqrt(np.float32(D)))


def build_body(ctx: ExitStack, tc: tile.TileContext, x, wa, wp, out, T: int):
    nc = tc.nc
    TT = T // 128              # number of 128-row t-tiles
    W = min(512, T)            # m-chunk width
    MCN = T // W               # number of m-chunks
    WT = W // 128              # n-tiles per m-chunk width

    const = ctx.enter_context(tc.tile_pool(name="const", bufs=1))
    wpool = ctx.enter_context(tc.tile_pool(name="wpool", bufs=1))
    wstage = ctx.enter_context(tc.tile_pool(name="wstage", bufs=4))
    # fresh slot per x t-tile: keeps every HWDGE input DMA at zero waits
    xload = ctx.enter_context(tc.tile_pool(name="xload", bufs=TT))
    xbfp = ctx.enter_context(tc.tile_pool(name="xbfp", bufs=3))
    xTp = ctx.enter_context(tc.tile_pool(name="xTp", bufs=1))
    qkTp = ctx.enter_context(tc.tile_pool(name="qkTp", bufs=1))
    vaugp = ctx.enter_context(tc.tile_pool(name="vaugp", bufs=TT))
    ptp = ctx.enter_context(tc.tile_pool(name="ptp", bufs=2))
    ytp = ctx.enter_context(tc.tile_pool(name="ytp", bufs=1))
    ysbp = ctx.enter_context(tc.tile_pool(name="ysbp", bufs=2))
    nrmp = ctx.enter_context(tc.tile_pool(name="nrmp", bufs=2))

    # PSUM budget (8 banks of [128, 2KB]):
    #   ps_s: 2 tags x 1 buf x [128, 1024] fp32 = 4 banks (score halves)
    #   ps_y: 2 tags x 1 buf x [128, 512] fp32 = 2 banks (y_aug accum)
    #   ps_sm: 1 shared tag x 2 bufs x <=1 bank = 2 banks
    ps_s = ctx.enter_context(tc.tile_pool(name="ps_s", bufs=1, space="PSUM"))
    ps_y = ctx.enter_context(tc.tile_pool(name="ps_y", bufs=1, space="PSUM"))
    ps_sm = ctx.enter_context(tc.tile_pool(name="ps_sm", bufs=2, space="PSUM"))

    # --- constants: triangular keep-mask (n' <= m'), replicated 2x ---
    tri = const.tile([128, 128], BF16)
    make_upper_triangular(nc, tri[:], val=1.0, diag=True)
    tri2 = const.tile([128, 256], BF16)
    for i in range(2):
        nc.vector.tensor_copy(tri2[:, 128 * i:128 * (i + 1)], tri[:])
    tri2v = tri2[:].rearrange("p (h m) -> p h m", h=2)
    ones128 = const.tile([128, D], BF16)
    nc.gpsimd.memset(ones128[:], 1.0)
    ident = const.tile([128, 128], BF16)
    make_identity(nc, ident[:])

    # --- weights: load fp32, cast to bf16 ---
    wa_bf = []
    wp_bf = []
    for k in range(2):
        wa_f = wstage.tile([128, 3 * C], FP32, name=f"wa_f{k}", tag="wstage")
        nc.sync.dma_start(wa_f[:], wa[128 * k:128 * (k + 1), :])
        wab = wpool.tile([128, 3 * C], BF16, name=f"wa_bf{k}")
        nc.vector.tensor_copy(wab[:], wa_f[:])
        wa_bf.append(wab)
        wp_f = wstage.tile([128, C], FP32, name=f"wp_f{k}", tag="wstage")
        nc.sync.dma_start(wp_f[:], wp[128 * k:128 * (k + 1), :])
        wpb = wpool.tile([128, C], BF16, name=f"wp_bf{k}")
        nc.vector.tensor_copy(wpb[:], wp_f[:])
        wp_bf.append(wpb)

    xT = [xTp.tile([128, T], BF16, name=f"xT{k}") for k in range(2)]
    qkT = [qkTp.tile([128, T], BF16, name=f"qkT{f}") for f in range(4)]
    yT = [ytp.tile([128, T], BF16, name=f"yT{g}") for g in range(2)]
    vaug = [None] * TT

    def prologue(c):
        # x tiles of chunk c: load, cast bf16, PE-transpose into xT cols
        for tt in range(WT * c, WT * (c + 1)):
            x_f = xload.tile([128, C], FP32, name="x_f")
            nc.sync.dma_start(x_f[:], x[128 * tt:128 * (tt + 1), :])
            x_bf = xbfp.tile([128, C], BF16, name="x_bf")
            nc.vector.tensor_copy(x_bf[:], x_f[:])
            for k in range(2):
                tp_ps = ps_sm.tile([128, 128], BF16, name="tp_ps", tag="sm")
                nc.tensor.transpose(tp_ps[:], x_bf[:, 128 * k:128 * (k + 1)],
                                    ident[:])
                nc.vector.tensor_copy(xT[k][:, 128 * tt:128 * (tt + 1)],
                                      tp_ps[:])
        # qkT chunk c for all 4 f-tiles (q rows 0:256 f=0,1; k rows 256:512)
        for f in range(4):
            ps = ps_sm.tile([128, W], FP32, name="qk_ps", tag="sm")
            for k in range(2):
                nc.tensor.matmul(
                    ps[:],
                    wa_bf[k][:, 128 * f:128 * (f + 1)],
                    xT[k][:, W * c:W * (c + 1)],
                    start=(k == 0),
                    stop=(k == 1),
                )
            nc.vector.tensor_copy(qkT[f][:, W * c:W * (c + 1)], ps[:])
        # v natural + ones column -> v_aug [128, 8*33] per t-tile
        for tt in range(WT * c, WT * (c + 1)):
            ps = ps_sm.tile([128, C], FP32, name="v_ps", tag="sm")
            for k in range(2):
                nc.tensor.matmul(
                    ps[:],
                    xT[k][:, 128 * tt:128 * (tt + 1)],
                    wa_bf[k][:, 2 * C:3 * C],
                    start=(k == 0),
                    stop=(k == 1),
                )
            va = vaugp.tile([128, H * (D + 1)], BF16, name="va")
            nc.gpsimd.memset(va[:], 1.0)
            nc.vector.tensor_copy(
                va[:].rearrange("p (h d) -> p h d", h=H)[:, :, 0:D],
                ps[:].rearrange("p (h d) -> p h d", h=H),
            )
            vaug[tt] = va

    def emit_pv(g, pts, off, nn, y_ps, start, stop):
        # y_aug^T[33, m] accumulation: head hloc=2b+i from pt half b slot i
        for b in range(2):
            for i in range(2):
                hg = 4 * g + 2 * b + i
                nc.tensor.matmul(
                    y_ps[b][ds(64 * i, D + 1), ds(off, W - off)],
                    vaug[nn][:, (D + 1) * hg:(D + 1) * hg + (D + 1)],
                    pts[b][:, W * i + off:W * (i + 1)],
                    start=start,
                    stop=stop,
                    tile_position=(0, 64 * i),
                    skip_group_check=True,
                )

    def attention(mc):
        for g in range(2):
            qt = qkT[g]       # q features for heads 4g..4g+3
            kt = qkT[2 + g]   # k features
            nn_count = WT * (mc + 1)
            nn_last = nn_count - 1
            y_ps = [ps_y.tile([128, W], FP32, name=f"y_ps{b}")
                    for b in range(2)]
            prev = None
            for nn in range(nn_count):
                j = nn - WT * mc  # >= 0 on diagonal-crossing tiles
                off = max(0, 128 * j)
                # S matmuls, split into two 2-head halves so the WAR on the
                # score PSUM resolves per-half (next S overlaps 2nd ACT)
                s_half = []
                for half in range(2):
                    s_ps = ps_s.tile([128, 2 * W], FP32, name=f"s_ps{half}",
                                     tag=f"s{half}")
                    for hi in range(2):
                        hh = 2 * half + hi
                        nc.tensor.matmul(
                            s_ps[:, W * hi:W * (hi + 1)],
                            kt[32 * hh:32 * (hh + 1),
                               128 * nn:128 * (nn + 1)],
                            qt[32 * hh:32 * (hh + 1), W * mc:W * (mc + 1)],
                            start=True,
                            stop=True,
                            tile_position=(32 * hh, 0),
                            skip_group_check=True,
                        )
                    s_half.append(s_ps)
                # previous step's PV is emitted AFTER this step's S matmuls
                # so the PE FIFO services S (the ACT chain) first
                if prev is not None:
                    emit_pv(g, prev[0], prev[1], prev[2], y_ps,
                            start=(prev[2] == 0), stop=False)
                # exp ACT per half; triangular mask on diagonal blocks
                pt_half = []
                for half in range(2):
                    pt = ptp.tile([128, 2 * W], BF16, name=f"pt{half}",
                                  tag=f"pt{half}")
                    sv = s_half[half][:].rearrange("p (h m) -> p h m", h=2)
                    pv = pt[:].rearrange("p (h m) -> p h m", h=2)
                    nc.scalar.activation(
                        pv[:, :, ds(off, W - off)],
                        sv[:, :, ds(off, W - off)],
                        mybir.ActivationFunctionType.Exp,
                        scale=SCALE,
                    )
                    if j >= 0:
                        blk = pv[:, :, ds(off, 128)]
                        nc.vector.tensor_mul(blk, blk, tri2v[:, :, 0:128])
                    pt_half.append(pt)
                prev = (pt_half, off, nn)
            emit_pv(g, prev[0], prev[1], prev[2], y_ps,
                    start=(prev[2] == 0), stop=True)

            # free the y PSUM banks immediately: copy y_aug bands to SBUF
            # (one [33, W] base-0 tile per head so later DVE ops have
            # matching start partitions - the BIR verifier requires SBUF
            # accesses to start 32-aligned / inputs partition-matched)
            y_sb = []
            for b in range(2):
                for i in range(2):
                    ys = ysbp.tile([D + 1, W], FP32, name=f"ysb{2 * b + i}",
                                   tag=f"ysb{2 * b + i}")
                    nc.vector.tensor_copy(ys[:], y_ps[b][ds(64 * i, D + 1), :])
                    y_sb.append(ys)

            # normalization (runs under later ACTs): gather denom rows to
            # 32-aligned partitions, ONE batched reciprocal (free-dim cost:
            # 8x cheaper than 4x [1,W]), Pool partition-broadcast (fp32
            # exact, no PE/PSUM), DVE mult -> yT bf16
            dg = nrmp.tile([128, W], FP32, name="dg", tag="dg")
            nc.gpsimd.memset(dg[:], 1.0)
            for hloc in range(4):
                nc.vector.tensor_copy(dg[32 * hloc:32 * hloc + 1, :],
                                      y_sb[hloc][D:D + 1, :])
            rcp = nrmp.tile([128, W], FP32, name="rcp", tag="rcp")
            nc.vector.reciprocal(rcp[:], dg[:])
            # batched hi/lo bf16 split (full precision via r = hi + lo),
            # then K=1 ones-matmul broadcast to 32 partitions per head
            r_hi = nrmp.tile([128, W], BF16, name="r_hi", tag="r_hi")
            nc.vector.tensor_copy(r_hi[:], rcp[:])
            r_lo = nrmp.tile([128, W], BF16, name="r_lo", tag="r_lo")
            nc.vector.tensor_sub(r_lo[:], rcp[:], r_hi[:])
            # PE operands must start at partition {0,32,64}: head 3's rows
            # (partition 96) need a base-0 staging copy first
            rf3 = nrmp.tile([1, W], FP32, name="rf3", tag="rf3")
            nc.vector.tensor_copy(rf3[:], rcp[96:97, :])
            hi3 = nrmp.tile([1, W], BF16, name="hi3", tag="hi3")
            nc.vector.tensor_copy(hi3[:], rf3[:])
            lo3 = nrmp.tile([1, W], BF16, name="lo3", tag="lo3")
            nc.vector.tensor_sub(lo3[:], rf3[:], hi3[:])
            for hloc in range(4):
                if hloc < 3:
                    hi_ap = r_hi[32 * hloc:32 * hloc + 1, :]
                    lo_ap = r_lo[32 * hloc:32 * hloc + 1, :]
                    ones_ap = ones128[32 * hloc:32 * hloc + 1, :]
                else:
                    hi_ap = hi3[:]
                    lo_ap = lo3[:]
                    ones_ap = ones128[0:1, :]
                bc_ps = ps_sm.tile([D, W], FP32, name="bc_ps", tag="sm")
                nc.tensor.matmul(bc_ps[:], ones_ap, hi_ap,
                                 start=True, stop=False)
                nc.tensor.matmul(bc_ps[:], ones_ap, lo_ap,
                                 start=False, stop=True)
                bcast = nrmp.tile([D, W], FP32, name="bcast", tag="bcast")
                nc.vector.tensor_copy(bcast[:], bc_ps[:])
                nc.vector.tensor_mul(
                    yT[g][32 * hloc:32 * hloc + 32, W * mc:W * (mc + 1)],
                    y_sb[hloc][0:D, :],
                    bcast[:],
                )
        # projection for this m-chunk's t-tiles
        for tt in range(WT * mc, WT * (mc + 1)):
            ps = ps_sm.tile([128, C], FP32, name="pj_ps", tag="sm")
            for g in range(2):
                nc.tensor.matmul(
                    ps[:],
                    yT[g][:, 128 * tt:128 * (tt + 1)],
                    wp_bf[g][:],
                    start=(g == 0),
                    stop=(g == 1),
                )
            ost = xbfp.tile([128, C], FP32, name="ost", tag="ost")
            nc.vector.tensor_copy(ost[:], ps[:])
            # SWDGE: out-store waits are executed by Q7 ucode
            nc.gpsimd.dma_start(out[128 * tt:128 * (tt + 1), :], ost[:])

    # emission schedule: prologue chunk c+1 overlaps attention chunk c
    prologue(0)
    if MCN > 1:
        prologue(1)
    for mc in range(MCN):
        attention(mc)
        if mc + 2 < MCN:
            prologue(mc + 2)


def build_nc(T: int = 2048) -> bass.Bass:
    # Bacc (not raw Bass): its compile() pass legalizes multi-sem waits via
    # event semaphores - walrus only accepts one sem wait per instruction.
    nc = bacc.Bacc("TRN2", target_bir_lowering=False, debug=False,
                   num_devices=N_CORES)
    x_d = nc.dram_tensor("x", [T, C], FP32, kind="ExternalInput")
    wa_d = nc.dram_tensor("w_attn", [C, 3 * C], FP32, kind="ExternalInput")
    wp_d = nc.dram_tensor("w_proj", [C, C], FP32, kind="ExternalInput")
    out_d = nc.dram_tensor("out", [T, C], FP32, kind="ExternalOutput")
    with tile.TileContext(nc) as tc:
        with ExitStack() as ctx:
            build_body(ctx, tc, x_d.ap(), wa_d.ap(), wp_d.ap(), out_d.ap(), T)
    nc.compile()
    return nc


_NC_CACHE: dict[int, bass.Bass] = {}


def _get_nc(T: int) -> bass.Bass:
    if T not in _NC_CACHE:
        _NC_CACHE[T] = build_nc(T)
    return _NC_CACHE[T]


def kernel(x: np.ndarray, w_attn: np.ndarray, w_proj: np.ndarray,
           **run_kwargs) -> np.ndarray:
    B, T, C_ = x.shape
    assert B == N_CORES and C_ == C
    nc = _get_nc(T)
    wa = np.ascontiguousarray(w_attn, dtype=np.float32)
    wp = np.ascontiguousarray(w_proj, dtype=np.float32)
    in_maps = [
        {"x": np.ascontiguousarray(x[b], dtype=np.float32), "w_attn": wa,
         "w_proj": wp}
        for b in range(B)
    ]
    res = run_bass_kernel_spmd(nc, in_maps, list(range(N_CORES)), **run_kwargs)
    out = np.stack([res.results[b]["out"] for b in range(B)])
    return out.astype(np.float32)


# revision 8
# speedup vs baseline: 1.6554x; 1.0098x over previous
"""Causal self-attention (B=8, T=2048, C=256, H=8, D=32) on 8 trn2 NeuronCores.

Sharding: pure data-parallel over batch - core b computes batch element b
end-to-end (no collectives).

Per-core kernel v2 - restructured so the scalar (ACT) engine, which owns the
irreducible exp work (~116us of element time), runs at ~100% duty:

  1. Pipelined prologue: x is processed in 512-column chunks (load, cast,
     PE-transpose, qkT chunk, vaug tiles), emitted so chunk c+1's prologue
     overlaps chunk c's attention.
  2. Attention in S^T layout per (head-group g of 4, m-chunk of 512):
     the score PSUM is split into TWO 2-bank tiles (heads 0-1 / heads 2-3)
     and the softmax exp ACT is split into matching halves, so the next
     n-tile's S matmuls (WAR on the score PSUM) can start as soon as the
     first half's ACT is done, hiding the S matmuls + semaphore handoff
     under the second ACT half. PV matmuls for step nn are emitted AFTER
     step nn+1's S matmuls so the PE queue never blocks the S->ACT chain.
  3. Causal masking: above-diagonal n-tiles skipped; diagonal 128-blocks
     get a triangular bf16 mask-multiply on the exp output (DVE, off the
     critical chain).
  4. PV via the 33-column v_aug trick (32 v-cols + ones column accumulates
     the softmax denominator) into 2 PSUM banks; after the last n-tile the
     y_aug PSUM is copied to SBUF immediately (frees banks for the next
     group) and normalization runs there: per-group batched reciprocal
     [4,512] (one instr instead of 4x [1,512] - 8x less DVE time),
     hi/lo-split K=1 broadcast matmuls, DVE multiply into yT bf16.
  5. proj per t-tile -> PSUM -> SWDGE DMA to DRAM, emitted per chunk.
"""

import numpy as np
from contextlib import ExitStack

import concourse.bass as bass
import concourse.bacc as bacc
import concourse.mybir as mybir
import concourse.tile as tile
from concourse.bass import ds
from concourse.bass_utils import run_bass_kernel_spmd
from concourse.masks import make_upper_triangular, make_identity
from concourse import library_config

FP32 = mybir.dt.float32
BF16 = mybir.dt.bfloat16

C = 256
H = 8
D = 32
N_CORES = 8
SCALE = 1.0 / float(np.sqrt(np.float32(D)))


def build_body(ctx: ExitStack, tc: tile.TileContext, x, wa, wp, out, T: int):
    nc = tc.nc
    TT = T // 128              # number of 128-row t-tiles
    W = min(512, T)            # m-chunk width
    MCN = T // W               # number of m-chunks
    WT = W // 128              # n-tiles per m-chunk width

    const = ctx.enter_context(tc.tile_pool(name="const", bufs=1))
    wpool = ctx.enter_context(tc.tile_pool(name="wpool", bufs=1))
    wstage = ctx.enter_context(tc.tile_pool(name="wstage", bufs=4))
    # fresh slot per x t-tile: keeps every HWDGE input DMA at zero waits
    xload = ctx.enter_context(tc.tile_pool(name="xload", bufs=TT))
    xbfp = ctx.enter_context(tc.tile_pool(name="xbfp", bufs=3))
    xTp = ctx.enter_context(tc.tile_pool(name="xTp", bufs=1))
    qkTp = ctx.enter_context(tc.tile_pool(name="qkTp", bufs=1))
    vaugp = ctx.enter_context(tc.tile_pool(name="vaugp", bufs=TT))
    ptp = ctx.enter_context(tc.tile_pool(name="ptp", bufs=2))
    ytp = ctx.enter_context(tc.tile_pool(name="ytp", bufs=1))
    ysbp = ctx.enter_context(tc.tile_pool(name="ysbp", bufs=2))
    nrmp = ctx.enter_context(tc.tile_pool(name="nrmp", bufs=2))

    # PSUM budget (8 banks of [128, 2KB]):
    #   ps_s: 2 tags x 1 buf x [128, 1024] fp32 = 4 banks (score halves)
    #   ps_y: 2 tags x 1 buf x [128, 512] fp32 = 2 banks (y_aug accum)
    #   ps_sm: 1 shared tag x 2 bufs x <=1 bank = 2 banks
    ps_s = ctx.enter_context(tc.tile_pool(name="ps_s", bufs=1, space="PSUM"))
    ps_y = ctx.enter_context(tc.tile_pool(name="ps_y", bufs=1, space="PSUM"))
    ps_sm = ctx.enter_context(tc.tile_pool(name="ps_sm", bufs=2, space="PSUM"))

    # --- constants: triangular keep-mask (n' <= m'), replicated 2x ---
    tri = const.tile([128, 128], BF16)
    make_upper_triangular(nc, tri[:], val=1.0, diag=True)
    tri2 = const.tile([128, 256], BF16)
    for i in range(2):
        nc.vector.tensor_copy(tri2[:, 128 * i:128 * (i + 1)], tri[:])
    tri2v = tri2[:].rearrange("p (h m) -> p h m", h=2)
    ones128 = const.tile([128, D], BF16)
    nc.gpsimd.memset(ones128[:], 1.0)
    ident = const.tile([128, 128], BF16)
    make_identity(nc, ident[:])

    # --- weights: load fp32, cast to bf16 ---
    wa_bf = []
    wp_bf = []
    for k in range(2):
        wa_f = wstage.tile([128, 3 * C], FP32, name=f"wa_f{k}", tag="wstage")
        nc.sync.dma_start(wa_f[:], wa[128 * k:128 * (k + 1), :])
        wab = wpool.tile([128, 3 * C], BF16, name=f"wa_bf{k}")
        nc.vector.tensor_copy(wab[:], wa_f[:])
        wa_bf.append(wab)
        wp_f = wstage.tile([128, C], FP32, name=f"wp_f{k}", tag="wstage")
        nc.sync.dma_start(wp_f[:], wp[128 * k:128 * (k + 1), :])
        wpb = wpool.tile([128, C], BF16, name=f"wp_bf{k}")
        nc.vector.tensor_copy(wpb[:], wp_f[:])
        wp_bf.append(wpb)

    xT = [xTp.tile([128, T], BF16, name=f"xT{k}") for k in range(2)]
    qkT = [qkTp.tile([128, T], BF16, name=f"qkT{f}") for f in range(4)]
    yT = [ytp.tile([128, T], BF16, name=f"yT{g}") for g in range(2)]
    vaug = [None] * TT

    xf_tiles = {}

    def pro_dma(c):
        # prefetch chunk c's x t-tiles (HWDGE, zero-wait via fresh slots)
        for tt in range(WT * c, WT * (c + 1)):
            x_f = xload.tile([128, C], FP32, name="x_f")
            nc.sync.dma_start(x_f[:], x[128 * tt:128 * (tt + 1), :])
            xf_tiles[tt] = x_f

    def pro_tt(tt):
        # cast bf16 + PE-transpose one x t-tile into xT columns
        x_f = xf_tiles.pop(tt)
        x_bf = xbfp.tile([128, C], BF16, name="x_bf")
        nc.vector.tensor_copy(x_bf[:], x_f[:])
        for k in range(2):
            tp_ps = ps_sm.tile([128, 128], BF16, name="tp_ps", tag="sm")
            nc.tensor.transpose(tp_ps[:], x_bf[:, 128 * k:128 * (k + 1)],
                                ident[:])
            nc.vector.tensor_copy(xT[k][:, 128 * tt:128 * (tt + 1)],
                                  tp_ps[:])

    def pro_qk(c, fpair):
        # qkT chunk c for an f-pair (q rows 0:256 f=0,1; k rows 256:512)
        for f in (2 * fpair, 2 * fpair + 1):
            ps = ps_sm.tile([128, W], FP32, name="qk_ps", tag="sm")
            for k in range(2):
                nc.tensor.matmul(
                    ps[:],
                    wa_bf[k][:, 128 * f:128 * (f + 1)],
                    xT[k][:, W * c:W * (c + 1)],
                    start=(k == 0),
                    stop=(k == 1),
                )
            nc.vector.tensor_copy(qkT[f][:, W * c:W * (c + 1)], ps[:])

    def pro_v(tt):
        # v natural + ones column -> v_aug [128, 8*33] for one t-tile
        ps = ps_sm.tile([128, C], FP32, name="v_ps", tag="sm")
        for k in range(2):
            nc.tensor.matmul(
                ps[:],
                xT[k][:, 128 * tt:128 * (tt + 1)],
                wa_bf[k][:, 2 * C:3 * C],
                start=(k == 0),
                stop=(k == 1),
            )
        va = vaugp.tile([128, H * (D + 1)], BF16, name="va")
        nc.gpsimd.memset(va[:], 1.0)
        nc.vector.tensor_copy(
            va[:].rearrange("p (h d) -> p h d", h=H)[:, :, 0:D],
            ps[:].rearrange("p (h d) -> p h d", h=H),
        )
        vaug[tt] = va

    def pro_units(c):
        # 8 small PE units, each < ~1.5us: interleave between attention
        # steps so the next chunk's prologue never blocks the PE FIFO
        u = [lambda tt=tt: pro_tt(tt) for tt in range(WT * c, WT * (c + 1))]
        u += [lambda fp=fp: pro_qk(c, fp) for fp in range(2)]
        u += [lambda t0=WT * c + 2 * p: (pro_v(t0), pro_v(t0 + 1))
              for p in range(2)]
        return u

    def emit_pv(g, pts, off, nn, y_ps, start, stop):
        # y_aug^T[33, m] accumulation: head hloc=2b+i from pt half b slot i
        for b in range(2):
            for i in range(2):
                hg = 4 * g + 2 * b + i
                nc.tensor.matmul(
                    y_ps[b][ds(64 * i, D + 1), ds(off, W - off)],
                    vaug[nn][:, (D + 1) * hg:(D + 1) * hg + (D + 1)],
                    pts[b][:, W * i + off:W * (i + 1)],
                    start=start,
                    stop=stop,
                    tile_position=(0, 64 * i),
                    skip_group_check=True,
                )

    def normalize(g, mc, y_ps):
        # free the y PSUM banks immediately: copy y_aug bands to SBUF
        # (one [33, W] base-0 tile per head so later DVE ops have matching
        # start partitions - BIR requires 32-aligned SBUF access starts)
        y_sb = []
        for b in range(2):
            for i in range(2):
                ys = ysbp.tile([D + 1, W], FP32, name=f"ysb{2 * b + i}",
                               tag=f"ysb{2 * b + i}")
                nc.vector.tensor_copy(ys[:], y_ps[b][ds(64 * i, D + 1), :])
                y_sb.append(ys)
        # gather denom rows to 32-aligned partitions, ONE batched
        # reciprocal (free-dim cost: 8x cheaper than 4x [1,W])
        dg = nrmp.tile([128, W], FP32, name="dg", tag="dg")
        nc.gpsimd.memset(dg[:], 1.0)
        for hloc in range(4):
            nc.vector.tensor_copy(dg[32 * hloc:32 * hloc + 1, :],
                                  y_sb[hloc][D:D + 1, :])
        rcp = nrmp.tile([128, W], FP32, name="rcp", tag="rcp")
        nc.vector.reciprocal(rcp[:], dg[:])
        # batched hi/lo bf16 split (full precision via r = hi + lo),
        # then K=1 ones-matmul broadcast to 32 partitions per head
        r_hi = nrmp.tile([128, W], BF16, name="r_hi", tag="r_hi")
        nc.vector.tensor_copy(r_hi[:], rcp[:])
        r_lo = nrmp.tile([128, W], BF16, name="r_lo", tag="r_lo")
        nc.vector.tensor_sub(r_lo[:], rcp[:], r_hi[:])
        # PE operands must start at partition {0,32,64}: head 3's rows
        # (partition 96) need a base-0 staging copy first
        rf3 = nrmp.tile([1, W], FP32, name="rf3", tag="rf3")
        nc.vector.tensor_copy(rf3[:], rcp[96:97, :])
        hi3 = nrmp.tile([1, W], BF16, name="hi3", tag="hi3")
        nc.vector.tensor_copy(hi3[:], rf3[:])
        lo3 = nrmp.tile([1, W], BF16, name="lo3", tag="lo3")
        nc.vector.tensor_sub(lo3[:], rf3[:], hi3[:])
        for hloc in range(4):
            if hloc < 3:
                hi_ap = r_hi[32 * hloc:32 * hloc + 1, :]
                lo_ap = r_lo[32 * hloc:32 * hloc + 1, :]
                ones_ap = ones128[32 * hloc:32 * hloc + 1, :]
            else:
                hi_ap = hi3[:]
                lo_ap = lo3[:]
                ones_ap = ones128[0:1, :]
            bc_ps = ps_sm.tile([D, W], FP32, name="bc_ps", tag="sm")
            nc.tensor.matmul(bc_ps[:], ones_ap, hi_ap,
                             start=True, stop=False)
            nc.tensor.matmul(bc_ps[:], ones_ap, lo_ap,
                             start=False, stop=True)
            bcast = nrmp.tile([D, W], FP32, name="bcast", tag="bcast")
            nc.vector.tensor_copy(bcast[:], bc_ps[:])
            nc.vector.tensor_mul(
                yT[g][32 * hloc:32 * hloc + 32, W * mc:W * (mc + 1)],
                y_sb[hloc][0:D, :],
                bcast[:],
            )

    def proj(mc):
        # projection for m-chunk mc's t-tiles; deferred into the NEXT
        # chunk's step stream so it never blocks the next S matmuls
        for tt in range(WT * mc, WT * (mc + 1)):
            ps = ps_sm.tile([128, C], FP32, name="pj_ps", tag="sm")
            for g in range(2):
                nc.tensor.matmul(
                    ps[:],
                    yT[g][:, 128 * tt:128 * (tt + 1)],
                    wp_bf[g][:],
                    start=(g == 0),
                    stop=(g == 1),
                )
            ost = xbfp.tile([128, C], FP32, name="ost", tag="ost")
            nc.vector.tensor_copy(ost[:], ps[:])
            # SWDGE: out-store waits are executed by Q7 ucode
            nc.gpsimd.dma_start(out[128 * tt:128 * (tt + 1), :], ost[:])

    def attention(mc, units=()):
        units = list(units)
        nn_count = WT * (mc + 1)
        nn_last = nn_count - 1
        y_ps_g = {}
        prev = None  # (g, pt_half, off, nn)
        for g, nn in [(g, nn) for g in range(2) for nn in range(nn_count)]:
            if nn == 0:
                y_ps_g[g] = [ps_y.tile([128, W], FP32, name=f"y_ps{b}")
                             for b in range(2)]
            qt = qkT[g]       # q features for heads 4g..4g+3
            kt = qkT[2 + g]   # k features
            j = nn - WT * mc  # >= 0 on diagonal-crossing tiles
            off = max(0, 128 * j)
            # S matmuls, split into two 2-head halves so the WAR on the
            # score PSUM resolves per-half (next S overlaps 2nd ACT)
            s_half = []
            for half in range(2):
                s_ps = ps_s.tile([128, 2 * W], FP32, name=f"s_ps{half}",
                                 tag=f"s{half}")
                for hi in range(2):
                    hh = 2 * half + hi
                    nc.tensor.matmul(
                        s_ps[:, W * hi:W * (hi + 1)],
                        kt[32 * hh:32 * (hh + 1), 128 * nn:128 * (nn + 1)],
                        qt[32 * hh:32 * (hh + 1), W * mc:W * (mc + 1)],
                        start=True,
                        stop=True,
                        tile_position=(32 * hh, 0),
                        skip_group_check=True,
                    )
                s_half.append(s_ps)
            # previous step's PV is emitted AFTER this step's S matmuls so
            # the PE FIFO services S (the ACT chain) first - including
            # across the g boundary; the finished group's normalize
            # follows (DVE-side, runs under subsequent ACTs)
            if prev is not None:
                pg, ppts, poff, pnn = prev
                emit_pv(pg, ppts, poff, pnn, y_ps_g[pg],
                        start=(pnn == 0), stop=(pnn == nn_last))
                if pnn == nn_last:
                    normalize(pg, mc, y_ps_g[pg])
            # exp ACT per half; triangular mask on diagonal blocks
            pt_half = []
            for half in range(2):
                pt = ptp.tile([128, 2 * W], BF16, name=f"pt{half}",
                              tag=f"pt{half}")
                sv = s_half[half][:].rearrange("p (h m) -> p h m", h=2)
                pv = pt[:].rearrange("p (h m) -> p h m", h=2)
                nc.scalar.activation(
                    pv[:, :, ds(off, W - off)],
                    sv[:, :, ds(off, W - off)],
                    mybir.ActivationFunctionType.Exp,
                    scale=SCALE,
                )
                if j >= 0:
                    blk = pv[:, :, ds(off, 128)]
                    nc.vector.tensor_mul(blk, blk, tri2v[:, :, 0:128])
                pt_half.append(pt)
            if units:
                units.pop(0)()
            prev = (g, pt_half, off, nn)
        pg, ppts, poff, pnn = prev
        emit_pv(pg, ppts, poff, pnn, y_ps_g[pg],
                start=(pnn == 0), stop=True)
        normalize(pg, mc, y_ps_g[pg])
        for u in units:
            u()

    # emission schedule: x prefetched up-front; chunk c+1's prologue and
    # chunk c-1's projection are interleaved into chunk c's step stream
    for c in range(MCN):
        pro_dma(c)
    for u in pro_units(0):
        u()
    pending = []
    for mc in range(MCN):
        units = pending + (pro_units(mc + 1) if mc + 1 < MCN else [])
        pending = [lambda m=mc: proj(m)]
        attention(mc, units)
    for u in pending:
        u()


def build_nc(T: int = 2048) -> bass.Bass:
    # Bacc (not raw Bass): its compile() pass legalizes multi-sem waits via
    # event semaphores - walrus only accepts one sem wait per instruction.
    nc = bacc.Bacc("TRN2", target_bir_lowering=False, debug=False,
                   num_devices=N_CORES)
    x_d = nc.dram_tensor("x", [T, C], FP32, kind="ExternalInput")
    wa_d = nc.dram_tensor("w_attn", [C, 3 * C], FP32, kind="ExternalInput")
    wp_d = nc.dram_tensor("w_proj", [C, C], FP32, kind="ExternalInput")
    out_d = nc.dram_tensor("out", [T, C], FP32, kind="ExternalOutput")
    with tile.TileContext(nc) as tc:
        with ExitStack() as ctx:
            build_body(ctx, tc, x_d.ap(), wa_d.ap(), wp_d.ap(), out_d.ap(), T)
    nc.compile()
    return nc


_NC_CACHE: dict[int, bass.Bass] = {}


def _get_nc(T: int) -> bass.Bass:
    if T not in _NC_CACHE:
        _NC_CACHE[T] = build_nc(T)
    return _NC_CACHE[T]


def kernel(x: np.ndarray, w_attn: np.ndarray, w_proj: np.ndarray,
           **run_kwargs) -> np.ndarray:
    B, T, C_ = x.shape
    assert B == N_CORES and C_ == C
    nc = _get_nc(T)
    wa = np.ascontiguousarray(w_attn, dtype=np.float32)
    wp = np.ascontiguousarray(w_proj, dtype=np.float32)
    in_maps = [
        {"x": np.ascontiguousarray(x[b], dtype=np.float32), "w_attn": wa,
         "w_proj": wp}
        for b in range(B)
    ]
    res = run_bass_kernel_spmd(nc, in_maps, list(range(N_CORES)), **run_kwargs)
    out = np.stack([res.results[b]["out"] for b in range(B)])
    return out.astype(np.float32)
